# revision 18
# baseline (speedup 1.0000x reference)
"""Trainium2 Bass kernel for nn_DebiasIntraDist (segment_reduce).

Full-input contract: kernel(**inputs) takes the complete (unsharded) inputs
and returns the full scalar loss. The N=65536 samples are sharded across the
8 NeuronCores by (demog, label-half): core 2d+h gets the rows with
demog == d and label-half h. Every core then owns a disjoint set of 256
(demog, label) groups, so no cross-core reduction is needed on device at
all: each core emits a (num, den) partial over its groups and the host
combines the 16 scalars into the final loss (the "gather/unshard" step).

Within each shard the rows are ordered so that all rows whose group falls
in PSUM chunk 0 (local label < 128) come first, then chunk 1. Each 128-row
tile therefore feeds exactly ONE [128-group x D] PSUM accumulator, halving
the matmul stream count vs. an unsorted layout.

Math per core, per group g:
    cnt[g], sums[g, :] (one-hot matmul), sumsq[g] = sum_i ||x_i||^2
    sum_{i in g} ||x_i - mu_g||^2 = sumsq[g] - ||sums[g]||^2 / cnt[g]

The sums matmul runs in a single bf16 pass: the stationary one-hot is
exact, the moving operand is the truncated-bf16 view of the fp32 rows
(free strided bitcast). Truncation acts as a ~(1 - 2^-8) multiplicative
shrink common to all groups, which cancels in the demog-deviation loss;
the residual noise lands ~5e-4 relative, far under the 2e-2 gate.

sumsq is exact: per-row Square+row-accumulate (rotated between the Scalar
and Vector engines) writes fp32 row norms into a [P, K] staging buffer.
Once per K-tile block, two vector ops split the whole block into bf16
(hi, lo) halves laid out as [hi K | lo K | ones K]; the per-tile 3-column
small matmul then reads a stride-K view [sq_hi, sq_lo, 1] and accumulates
exact [group-sumsq-hi, -lo, count] with the SAME bf16 one-hot as the big
matmul (one LDWEIGHTS dtype, no fp32 weight loads).
"""

import numpy as np

try:
    import concourse.bacc as bacc
except ImportError:  # fresh environment without PYTHONPATH set up
    import sys
    for p in ("/root/.axon_site/_ro/trn_rl_repo", "/opt/trn_rl_repo",
              "/root/.axon_site/_ro/pypackages"):
        if p not in sys.path:
            sys.path.append(p)
    import concourse.bacc as bacc
import concourse.mybir as mybir
import concourse.tile as tile
import concourse.bass_utils as bass_utils

N_CORES = 8
P = 128
D = 512          # feature dim
NL = 256         # labels per core after (demog, label-half) sharding
ND = 4           # demog values
NCH = NL // P    # one-hot chunks of 128 groups
CH = 8           # sample-tiles per feats DMA (2 MiB)
K = 8            # tiles per sq-split block
SQ_PATTERN = "svsvs"  # per-tile row-sumsq engine rotation (scalar/vector)

_cache: dict[tuple, object] = {}


def _build(key, debug: bool = False):
    """Compile the SPMD kernel for chunk tile counts (T0, T1)."""
    T0, T1 = key
    T = T0 + T1
    fp32 = mybir.dt.float32
    bf16 = mybir.dt.bfloat16
    Alu = mybir.AluOpType
    Act = mybir.ActivationFunctionType

    nc = bacc.Bacc("TRN2", target_bir_lowering=False, debug=False,
                   enable_asserts=False, num_devices=N_CORES)

    feats_t = nc.dram_tensor("feats_t", [P, T * D], fp32,
                             kind="ExternalInput").ap()
    labels_t = nc.dram_tensor("labels_t", [P, T], fp32,
                              kind="ExternalInput").ap()
    nd_out = nc.dram_tensor("nd", [1, 2], fp32, kind="ExternalOutput").ap()

    chunk_start = (0, T0)
    chunk_stop = (T0 - 1, T - 1)

    with tile.TileContext(nc) as tc:
        with (
            tc.tile_pool(name="const", bufs=1) as constp,
            tc.tile_pool(name="fx", bufs=4) as fxp,
            tc.tile_pool(name="oh16", bufs=K + 3) as oh16p,
            tc.tile_pool(name="scr", bufs=4) as scrp,
            tc.tile_pool(name="post", bufs=1) as postp,
            tc.tile_pool(name="ps", bufs=1, space="PSUM") as psp,
        ):
            # per-group accumulators; each PSUM accumulation group owns a bank
            ps_sums = [psp.tile([P, D], fp32, tag=f"sums{c}", name=f"sums{c}")
                       for c in range(NCH)]
            ps_small = [psp.tile([P, 3], fp32, tag=f"small{c}",
                                 name=f"small{c}")
                        for c in range(NCH)]

            # constants
            iota16 = constp.tile([P, NL], bf16, tag="iota16")
            nc.gpsimd.iota(iota16[:], [[1, NL]], channel_multiplier=0,
                           allow_small_or_imprecise_dtypes=True)
            labs = constp.tile([P, T], fp32, tag="labs")
            nc.scalar.dma_start(out=labs[:], in_=labels_t[:])

            # rotating sq staging blocks: sqb holds K fp32 row norms, r3b is
            # the bf16 [hi K | lo K | ones K] split (ones written once)
            NBLK = 3
            sqbs, r3bs = [], []
            for b in range(NBLK):
                sqb = constp.tile([P, K], fp32, tag=f"sqb{b}", name=f"sqb{b}")
                r3b = constp.tile([P, 3 * K], bf16, tag=f"r3b{b}",
                                  name=f"r3b{b}")
                nc.gpsimd.memset(r3b[:, 2 * K:3 * K], 1.0)
                sqbs.append(sqb)
                r3bs.append(r3b)
            ones = constp.tile([P, 1], fp32, tag="ones")
            nc.gpsimd.memset(ones[:], 1.0)

            # post-processing tiles (chunk c's slice filled as soon as its
            # accumulation closes, overlapping the other chunk's main loop)
            norm2 = postp.tile([P, NCH], fp32, tag="norm2")
            small = postp.tile([P, 3 * NCH], fp32, tag="small")

            def post_norm2(c):
                scr2 = scrp.tile([P, D], fp32, tag="pscr")
                nc.scalar.activation(scr2[:], ps_sums[c][:], Act.Square,
                                     accum_out=norm2[:, c:c + 1])

            def post_small(c):
                nc.vector.tensor_copy(out=small[:, 3 * c:3 * c + 3],
                                      in_=ps_small[c][:])

            def flush_block(blk, tis, ohs):
                """Split block blk's fp32 row norms into bf16 hi/lo and run
                the delayed small matmuls for its tiles."""
                sqb, r3b = sqbs[blk % NBLK], r3bs[blk % NBLK]
                kk = len(tis)
                hi = sqb[:].bitcast(bf16)[:, 1:2 * kk:2]
                nc.vector.tensor_copy(out=r3b[:, 0:kk], in_=hi)
                nc.vector.tensor_tensor(out=r3b[:, K:K + kk],
                                        in0=sqb[:, 0:kk],
                                        in1=r3b[:, 0:kk], op=Alu.subtract)
                for ti2, oh2 in zip(tis, ohs):
                    c2 = 0 if ti2 < T0 else 1
                    k2 = ti2 % K
                    r3 = r3b[:, k2:k2 + 2 * K + 1:K]  # [sq_hi, sq_lo, 1]
                    nc.tensor.matmul(out=ps_small[c2][:], lhsT=oh2[:],
                                     rhs=r3, start=ti2 in chunk_start,
                                     stop=ti2 in chunk_stop)
                    if ti2 in chunk_stop:
                        post_small(c2)

            t = 0
            first_chunk = True
            nchunk = 0
            blk_tis, blk_ohs = [], []
            while t < T:
                L = 1 if first_chunk else min(CH, T - t)
                first_chunk = False
                fx = fxp.tile([P, CH * D], fp32, tag="fx")
                # alternate the issuing queue so DMA issue latency overlaps
                dma_eng = nc.sync if nchunk % 2 == 0 else nc.gpsimd
                dma_eng.dma_start(out=fx[:, :L * D],
                                  in_=feats_t[:, t * D:(t + L) * D])
                nchunk += 1
                fxb = fx[:].bitcast(bf16)  # [P, CH*2D] uint16-granular view
                for j in range(L):
                    ti = t + j
                    c = 0 if ti < T0 else 1
                    blk = ti // K
                    X = fx[:, j * D:(j + 1) * D]
                    xhi = fxb[:, j * 2 * D + 1:(j + 1) * 2 * D:2]
                    # one-hot of this tile's labels vs the active group chunk
                    oh16 = oh16p.tile([P, P], bf16, tag="oh16")
                    nc.vector.tensor_scalar(
                        out=oh16[:], in0=iota16[:, c * P:(c + 1) * P],
                        scalar1=labs[:, ti:ti + 1], scalar2=None,
                        op0=Alu.is_equal)
                    # exact fp32 row sumsq, engine rotated
                    sqcol = sqbs[blk % NBLK][:, ti % K:ti % K + 1]
                    scr = scrp.tile([P, D], bf16, tag="scr")
                    if SQ_PATTERN[ti % len(SQ_PATTERN)] == "s":
                        nc.scalar.activation(scr[:], X, Act.Square,
                                             accum_out=sqcol)
                    else:
                        nc.vector.scalar_tensor_tensor(
                            out=scr[:], in0=X, scalar=1.0, in1=X,
                            op0=Alu.mult, op1=Alu.mult, accum_out=sqcol)
                    nc.tensor.matmul(out=ps_sums[c][:], lhsT=oh16[:],
                                     rhs=xhi, start=ti in chunk_start,
                                     stop=ti in chunk_stop)
                    if ti in chunk_stop:
                        post_norm2(c)
                    blk_tis.append(ti)
                    blk_ohs.append(oh16)
                    if ti % K == K - 1 or ti == T - 1:
                        flush_block(blk, blk_tis, blk_ohs)
                        blk_tis, blk_ohs = [], []
                t += L

            # final reduction of this core's 256 groups to (num, den)
            sumsq = postp.tile([P, NCH], fp32, tag="sumsq")
            nc.vector.tensor_tensor(out=sumsq[:], in0=small[:, 0::3],
                                    in1=small[:, 1::3], op=Alu.add)
            cnt = small[:, 2::3]  # [P, NCH]
            safe = postp.tile([P, NCH], fp32, tag="safe")
            nc.vector.tensor_scalar_max(safe[:], cnt, 1.0)
            inv = postp.tile([P, NCH], fp32, tag="inv")
            nc.vector.reciprocal(inv[:], safe[:])
            # grp = (sumsq - norm2 * inv) * inv
            t1 = postp.tile([P, NCH], fp32, tag="t1")
            nc.vector.tensor_tensor(out=t1[:], in0=norm2[:], in1=inv[:],
                                    op=Alu.mult)
            t2 = postp.tile([P, NCH], fp32, tag="t2")
            nc.vector.tensor_tensor(out=t2[:], in0=sumsq[:], in1=t1[:],
                                    op=Alu.subtract)
            grp = postp.tile([P, NCH], fp32, tag="grp")
            nc.vector.tensor_tensor(out=grp[:], in0=t2[:], in1=inv[:],
                                    op=Alu.mult)
            pres = postp.tile([P, NCH], fp32, tag="pres")
            nc.vector.tensor_scalar(out=pres[:], in0=cnt, scalar1=0.0,
                                    scalar2=None, op0=Alu.is_gt)
            # pack [grp*pres | pres]; reduce over this core's groups via PE
            pk = postp.tile([P, 2 * NCH], fp32, tag="pk")
            nc.vector.tensor_tensor(out=pk[:, 0:NCH], in0=grp[:],
                                    in1=pres[:], op=Alu.mult)
            nc.vector.tensor_copy(out=pk[:, NCH:2 * NCH], in_=pres[:])
            ps4 = psp.tile([1, 2 * NCH], fp32, tag="ps4")
            nc.tensor.matmul(out=ps4[:], lhsT=ones[:], rhs=pk[:],
                             start=True, stop=True)
            s4 = postp.tile([1, 2 * NCH], fp32, tag="s4")
            nc.vector.tensor_copy(out=s4[:], in_=ps4[:])
            nd_t = postp.tile([1, 2], fp32, tag="nd_t")
            nc.vector.tensor_reduce(out=nd_t[:, 0:1], in_=s4[:1, 0:NCH],
                                    axis=mybir.AxisListType.X, op=Alu.add)
            nc.vector.tensor_reduce(out=nd_t[:, 1:2], in_=s4[:1, NCH:2 * NCH],
                                    axis=mybir.AxisListType.X, op=Alu.add)
            nc.sync.dma_start(out=nd_out[:], in_=nd_t[:])

    nc.compile()
    return nc


def _shard(feats, labels, demog):
    """Partition rows by (demog, label-half) -> core 2d+h; within each core
    order rows by PSUM chunk (local label < 128 first), padding each chunk
    section to the compile-time tile counts (T0, T1)."""
    half = (labels >= NL).astype(np.int32)
    shard_id = demog * 2 + half
    loc = labels % NL
    chunk = loc // P
    parts = []  # per core: (rows_chunk0, rows_chunk1)
    for s in range(N_CORES):
        in_s = shard_id == s
        parts.append((np.flatnonzero(in_s & (chunk == 0)),
                      np.flatnonzero(in_s & (chunk == 1))))
    T0 = max(1, max(-(-len(p[0]) // P) for p in parts))
    T1 = max(1, max(-(-len(p[1]) // P) for p in parts))
    T = T0 + T1
    S = T * P
    in_maps = []
    for r0, r1 in parts:
        f = np.zeros((S, D), np.float32)
        lab = np.full(S, 999.0, np.float32)  # pad label matches no group
        f[:len(r0)] = feats[r0]
        lab[:len(r0)] = loc[r0]
        f[T0 * P:T0 * P + len(r1)] = feats[r1]
        lab[T0 * P:T0 * P + len(r1)] = loc[r1]
        # [S, D] -> [P, T*D]: partition p holds its rows contiguously so
        # every DMA descriptor is a fat contiguous run
        ft = np.ascontiguousarray(
            f.reshape(T, P, D).transpose(1, 0, 2).reshape(P, T * D))
        lt = np.ascontiguousarray(lab.reshape(T, P).T)
        in_maps.append({"feats_t": ft, "labels_t": lt})
    return (T0, T1), in_maps


def kernel(feats, labels, demog_labels, _results_out=None):
    feats = np.ascontiguousarray(np.asarray(feats), dtype=np.float32)
    labels = np.asarray(labels).astype(np.int32)
    demog = np.asarray(demog_labels).astype(np.int32)
    assert feats.ndim == 2 and feats.shape[1] == D

    key, in_maps = _shard(feats, labels, demog)
    nc = _cache.get(key)
    if nc is None:
        nc = _cache.setdefault(key, _build(key))
    res = None
    last_exc = None
    for attempt in range(3):
        try:
            res = bass_utils.run_bass_kernel_spmd(
                nc, in_maps, core_ids=list(range(N_CORES)))
            break
        except Exception as e:  # transient axon worker hangups
            last_exc = e
            import time
            time.sleep(10)
    if res is None:
        raise last_exc
    if _results_out is not None:
        _results_out.append(res)
    nds = np.stack([np.asarray(res.results[i]["nd"]).reshape(2)
                    for i in range(N_CORES)])  # [8, 2]
    num = nds[0::2, 0] + nds[1::2, 0]  # per-demog numerator
    den = nds[0::2, 1] + nds[1::2, 1]
    intra = num / np.maximum(den, 1.0)
    loss = np.mean(np.abs(intra - np.mean(intra)))
    return np.float32(loss)


# revision 25
# speedup vs baseline: 1.1840x; 1.1840x over previous
"""Trainium2 Bass kernel for nn_DebiasIntraDist (segment_reduce).

Full-input contract: kernel(**inputs) takes the complete (unsharded) inputs
and returns the full scalar loss. The N=65536 samples are sharded across the
8 NeuronCores by (demog, label-half): core 2d+h gets the rows with
demog == d and label-half h. Every core then owns a disjoint set of 256
(demog, label) groups, so no cross-core reduction is needed on device at
all: each core emits a (num, den) partial over its groups and the host
combines the 16 scalars into the final loss (the "gather/unshard" step).

Within each shard the rows are ordered so that all rows whose group falls
in PSUM chunk 0 (local label < 128) come first, then chunk 1. Each 128-row
tile therefore feeds exactly ONE [128-group x D] PSUM accumulator, halving
the matmul stream count vs. an unsorted layout.

Math per core, per group g:
    cnt[g], sums[g, :] (one-hot matmul), sumsq[g] = sum_i ||x_i||^2
    sum_{i in g} ||x_i - mu_g||^2 = sumsq[g] - ||sums[g]||^2 / cnt[g]

The sums matmul runs in a single bf16 pass: the stationary one-hot is
exact, the moving operand is the truncated-bf16 view of the fp32 rows
(free strided bitcast). Truncation acts as a ~(1 - 2^-8) multiplicative
shrink common to all groups, which cancels in the demog-deviation loss;
the residual noise lands ~5e-4 relative, far under the 2e-2 gate.

sumsq is exact: per-row Square+row-accumulate (rotated between the Scalar
and Vector engines) writes fp32 row norms into a [P, K] staging buffer.
Once per K-tile block, two vector ops split the whole block into bf16
(hi, lo) halves laid out as [hi K | lo K | ones K]; the per-tile 3-column
small matmul then reads a stride-K view [sq_hi, sq_lo, 1] and accumulates
exact [group-sumsq-hi, -lo, count] with the SAME bf16 one-hot as the big
matmul (one LDWEIGHTS dtype, no fp32 weight loads).
"""

import numpy as np

try:
    import concourse.bacc as bacc
except ImportError:  # fresh environment without PYTHONPATH set up
    import sys
    for p in ("/root/.axon_site/_ro/trn_rl_repo", "/opt/trn_rl_repo",
              "/root/.axon_site/_ro/pypackages"):
        if p not in sys.path:
            sys.path.append(p)
    import concourse.bacc as bacc
import concourse.mybir as mybir
import concourse.tile as tile
import concourse.bass_utils as bass_utils

N_CORES = 8
P = 128
D = 512          # feature dim
NL = 256         # labels per core after (demog, label-half) sharding
ND = 4           # demog values
NCH = NL // P    # one-hot chunks of 128 groups
CH = 6           # sample-tiles per feats DMA (1.5 MiB)
K = 8            # tiles per sq-split block
SQ_PATTERN = "svsvs"  # per-tile row-sumsq engine rotation (scalar/vector)

_cache: dict[tuple, object] = {}


def _build(key, debug: bool = False):
    """Compile the SPMD kernel for chunk tile counts (T0, T1)."""
    T0, T1 = key
    T = T0 + T1
    fp32 = mybir.dt.float32
    bf16 = mybir.dt.bfloat16
    Alu = mybir.AluOpType
    Act = mybir.ActivationFunctionType

    nc = bacc.Bacc("TRN2", target_bir_lowering=False, debug=False,
                   enable_asserts=False, num_devices=N_CORES)

    feats_t = nc.dram_tensor("feats_t", [P, T * D], fp32,
                             kind="ExternalInput").ap()
    # labels_t carries [labels | iota table] so the one-hot inputs arrive in
    # one early scalar-queue DMA with no gpsimd dependency
    labels_t = nc.dram_tensor("labels_t", [P, T + NL], fp32,
                              kind="ExternalInput").ap()
    nd_out = nc.dram_tensor("nd", [1, 2], fp32, kind="ExternalOutput").ap()

    chunk_start = (0, T0)
    chunk_stop = (T0 - 1, T - 1)

    with tile.TileContext(nc) as tc:
        with (
            tc.tile_pool(name="const", bufs=1) as constp,
            tc.tile_pool(name="fx", bufs=4) as fxp,
            tc.tile_pool(name="oh16", bufs=K + 3) as oh16p,
            tc.tile_pool(name="scr", bufs=4) as scrp,
            tc.tile_pool(name="post", bufs=1) as postp,
            tc.tile_pool(name="ps", bufs=1, space="PSUM") as psp,
            tc.tile_pool(name="dram", bufs=1, space="DRAM") as dramp,
        ):
            # per-group accumulators; each PSUM accumulation group owns a bank
            ps_sums = [psp.tile([P, D], fp32, tag=f"sums{c}", name=f"sums{c}")
                       for c in range(NCH)]
            ps_small = [psp.tile([P, 3], fp32, tag=f"small{c}",
                                 name=f"small{c}")
                        for c in range(NCH)]

            # constants: labels + iota table in one early DMA
            labs = constp.tile([P, T + NL], fp32, tag="labs")
            nc.scalar.dma_start(out=labs[:], in_=labels_t[:])
            iota32 = labs[:, T:T + NL]

            # rotating sq staging blocks: sqb holds K fp32 row norms, r3b is
            # the bf16 [hi K | lo K | ones K] split (ones written once)
            NBLK = 3
            sqbs, r3bs = [], []
            for b in range(NBLK):
                sqb = constp.tile([P, K], fp32, tag=f"sqb{b}", name=f"sqb{b}")
                r3b = constp.tile([P, 3 * K], bf16, tag=f"r3b{b}",
                                  name=f"r3b{b}")
                nc.gpsimd.memset(r3b[:, 2 * K:3 * K], 1.0)
                sqbs.append(sqb)
                r3bs.append(r3b)
            ones = constp.tile([P, 1], fp32, tag="ones")
            nc.gpsimd.memset(ones[:], 1.0)

            # post-processing tiles (chunk c's slice filled as soon as its
            # accumulation closes, overlapping the other chunk's main loop)
            norm2 = postp.tile([P, NCH], fp32, tag="norm2")
            small = postp.tile([P, 3 * NCH], fp32, tag="small")

            def post_norm2(c):
                scr2 = scrp.tile([P, D], fp32, tag="pscr")
                nc.scalar.activation(scr2[:], ps_sums[c][:], Act.Square,
                                     accum_out=norm2[:, c:c + 1])

            def post_small(c):
                nc.vector.tensor_copy(out=small[:, 3 * c:3 * c + 3],
                                      in_=ps_small[c][:])

            def flush_block(blk, tis, ohs):
                """Split block blk's fp32 row norms into bf16 hi/lo and run
                the delayed small matmuls for its tiles."""
                sqb, r3b = sqbs[blk % NBLK], r3bs[blk % NBLK]
                kk = len(tis)
                hi = sqb[:].bitcast(bf16)[:, 1:2 * kk:2]
                nc.vector.tensor_copy(out=r3b[:, 0:kk], in_=hi)
                nc.vector.tensor_tensor(out=r3b[:, K:K + kk],
                                        in0=sqb[:, 0:kk],
                                        in1=r3b[:, 0:kk], op=Alu.subtract)
                for ti2, oh2 in zip(tis, ohs):
                    c2 = 0 if ti2 < T0 else 1
                    k2 = ti2 % K
                    r3 = r3b[:, k2:k2 + 2 * K + 1:K]  # [sq_hi, sq_lo, 1]
                    nc.tensor.matmul(out=ps_small[c2][:], lhsT=oh2[:],
                                     rhs=r3, start=ti2 in chunk_start,
                                     stop=ti2 in chunk_stop)
                    if ti2 in chunk_stop:
                        post_small(c2)

            # DRAM scratch for warming the output-DMA path mid-loop
            warm_dram = dramp.tile([1, 2], fp32)

            t = 0
            first_chunk = True
            nchunk = 0
            warmed = False
            blk_tis, blk_ohs = [], []
            while t < T:
                L = 1 if first_chunk else min(CH, T - t)
                first_chunk = False
                fx = fxp.tile([P, CH * D], fp32, tag="fx")
                # the sync queue spends ~9us in the NEFF preamble; issue the
                # first chunks from the scalar queue which is free by ~2.5us
                dma_eng = nc.scalar if nchunk < 3 else nc.sync
                dma_eng.dma_start(out=fx[:, :L * D],
                                  in_=feats_t[:, t * D:(t + L) * D])
                nchunk += 1
                if not warmed and t + L >= T - 2 * CH:
                    # keep the output-DMA engine hot for the final nd store
                    nc.sync.dma_start(out=warm_dram[:], in_=labs[:1, :2])
                    warmed = True
                fxb = fx[:].bitcast(bf16)  # [P, CH*2D] uint16-granular view
                for j in range(L):
                    ti = t + j
                    c = 0 if ti < T0 else 1
                    blk = ti // K
                    X = fx[:, j * D:(j + 1) * D]
                    xhi = fxb[:, j * 2 * D + 1:(j + 1) * 2 * D:2]
                    # one-hot of this tile's labels vs the active group chunk
                    oh16 = oh16p.tile([P, P], bf16, tag="oh16")
                    nc.vector.tensor_scalar(
                        out=oh16[:], in0=iota32[:, c * P:(c + 1) * P],
                        scalar1=labs[:, ti:ti + 1], scalar2=None,
                        op0=Alu.is_equal)
                    # exact fp32 row sumsq, engine rotated
                    sqcol = sqbs[blk % NBLK][:, ti % K:ti % K + 1]
                    scr = scrp.tile([P, D], bf16, tag="scr")
                    if SQ_PATTERN[ti % len(SQ_PATTERN)] == "s":
                        nc.scalar.activation(scr[:], X, Act.Square,
                                             accum_out=sqcol)
                    else:
                        nc.vector.scalar_tensor_tensor(
                            out=scr[:], in0=X, scalar=1.0, in1=X,
                            op0=Alu.mult, op1=Alu.mult, accum_out=sqcol)
                    nc.tensor.matmul(out=ps_sums[c][:], lhsT=oh16[:],
                                     rhs=xhi, start=ti in chunk_start,
                                     stop=ti in chunk_stop)
                    if ti in chunk_stop:
                        post_norm2(c)
                    blk_tis.append(ti)
                    blk_ohs.append(oh16)
                    if ti % K == K - 1 or ti == T - 1:
                        flush_block(blk, blk_tis, blk_ohs)
                        blk_tis, blk_ohs = [], []
                t += L

            # final reduction of this core's 256 groups to (num, den)
            sumsq = postp.tile([P, NCH], fp32, tag="sumsq")
            nc.vector.tensor_tensor(out=sumsq[:], in0=small[:, 0::3],
                                    in1=small[:, 1::3], op=Alu.add)
            cnt = small[:, 2::3]  # [P, NCH]
            safe = postp.tile([P, NCH], fp32, tag="safe")
            nc.vector.tensor_scalar_max(safe[:], cnt, 1.0)
            inv = postp.tile([P, NCH], fp32, tag="inv")
            nc.vector.reciprocal(inv[:], safe[:])
            # grp = (sumsq - norm2 * inv) * inv
            t1 = postp.tile([P, NCH], fp32, tag="t1")
            nc.vector.tensor_tensor(out=t1[:], in0=norm2[:], in1=inv[:],
                                    op=Alu.mult)
            t2 = postp.tile([P, NCH], fp32, tag="t2")
            nc.vector.tensor_tensor(out=t2[:], in0=sumsq[:], in1=t1[:],
                                    op=Alu.subtract)
            grp = postp.tile([P, NCH], fp32, tag="grp")
            nc.vector.tensor_tensor(out=grp[:], in0=t2[:], in1=inv[:],
                                    op=Alu.mult)
            pres = postp.tile([P, NCH], fp32, tag="pres")
            nc.vector.tensor_scalar(out=pres[:], in0=cnt, scalar1=0.0,
                                    scalar2=None, op0=Alu.is_gt)
            # pack [grp*pres | pres]; reduce over this core's groups via PE
            pk = postp.tile([P, 2 * NCH], fp32, tag="pk")
            nc.vector.tensor_tensor(out=pk[:, 0:NCH], in0=grp[:],
                                    in1=pres[:], op=Alu.mult)
            nc.vector.tensor_copy(out=pk[:, NCH:2 * NCH], in_=pres[:])
            ps4 = psp.tile([1, 2 * NCH], fp32, tag="ps4")
            nc.tensor.matmul(out=ps4[:], lhsT=ones[:], rhs=pk[:],
                             start=True, stop=True)
            s4 = postp.tile([1, 2 * NCH], fp32, tag="s4")
            nc.vector.tensor_copy(out=s4[:], in_=ps4[:])
            nd_t = postp.tile([1, 2], fp32, tag="nd_t")
            nc.vector.tensor_reduce(out=nd_t[:, 0:1], in_=s4[:1, 0:NCH],
                                    axis=mybir.AxisListType.X, op=Alu.add)
            nc.vector.tensor_reduce(out=nd_t[:, 1:2], in_=s4[:1, NCH:2 * NCH],
                                    axis=mybir.AxisListType.X, op=Alu.add)
            nc.sync.dma_start(out=nd_out[:], in_=nd_t[:])

    nc.compile()
    return nc


def _shard(feats, labels, demog):
    """Partition rows by (demog, label-half) -> core 2d+h; within each core
    order rows by PSUM chunk (local label < 128 first), padding each chunk
    section to the compile-time tile counts (T0, T1)."""
    half = (labels >= NL).astype(np.int32)
    shard_id = demog * 2 + half
    loc = labels % NL
    chunk = loc // P
    parts = []  # per core: (rows_chunk0, rows_chunk1)
    for s in range(N_CORES):
        in_s = shard_id == s
        parts.append((np.flatnonzero(in_s & (chunk == 0)),
                      np.flatnonzero(in_s & (chunk == 1))))
    T0 = max(1, max(-(-len(p[0]) // P) for p in parts))
    T1 = max(1, max(-(-len(p[1]) // P) for p in parts))
    T = T0 + T1
    S = T * P
    in_maps = []
    for r0, r1 in parts:
        f = np.zeros((S, D), np.float32)
        lab = np.full(S, 999.0, np.float32)  # pad label matches no group
        f[:len(r0)] = feats[r0]
        lab[:len(r0)] = loc[r0]
        f[T0 * P:T0 * P + len(r1)] = feats[r1]
        lab[T0 * P:T0 * P + len(r1)] = loc[r1]
        # [S, D] -> [P, T*D]: partition p holds its rows contiguously so
        # every DMA descriptor is a fat contiguous run
        ft = np.ascontiguousarray(
            f.reshape(T, P, D).transpose(1, 0, 2).reshape(P, T * D))
        lt = np.ascontiguousarray(np.concatenate(
            [lab.reshape(T, P).T,
             np.tile(np.arange(NL, dtype=np.float32), (P, 1))], axis=1))
        in_maps.append({"feats_t": ft, "labels_t": lt})
    return (T0, T1), in_maps


def kernel(feats, labels, demog_labels, _results_out=None):
    feats = np.ascontiguousarray(np.asarray(feats), dtype=np.float32)
    labels = np.asarray(labels).astype(np.int32)
    demog = np.asarray(demog_labels).astype(np.int32)
    assert feats.ndim == 2 and feats.shape[1] == D

    key, in_maps = _shard(feats, labels, demog)
    nc = _cache.get(key)
    if nc is None:
        nc = _cache.setdefault(key, _build(key))
    res = None
    last_exc = None
    for attempt in range(3):
        try:
            res = bass_utils.run_bass_kernel_spmd(
                nc, in_maps, core_ids=list(range(N_CORES)))
            break
        except Exception as e:  # transient axon worker hangups
            last_exc = e
            import time
            time.sleep(10)
    if res is None:
        raise last_exc
    if _results_out is not None:
        _results_out.append(res)
    nds = np.stack([np.asarray(res.results[i]["nd"]).reshape(2)
                    for i in range(N_CORES)])  # [8, 2]
    num = nds[0::2, 0] + nds[1::2, 0]  # per-demog numerator
    den = nds[0::2, 1] + nds[1::2, 1]
    intra = num / np.maximum(den, 1.0)
    loss = np.mean(np.abs(intra - np.mean(intra)))
    return np.float32(loss)


# revision 29
# speedup vs baseline: 1.2498x; 1.0556x over previous
"""Trainium2 Bass kernel for nn_DebiasIntraDist (segment_reduce).

Full-input contract: kernel(**inputs) takes the complete (unsharded) inputs
and returns the full scalar loss. The N=65536 samples are sharded across the
8 NeuronCores by (demog, label-half): core 2d+h gets the rows with
demog == d and label-half h. Every core then owns a disjoint set of 256
(demog, label) groups, so no cross-core reduction is needed on device at
all: each core emits a (num, den) partial over its groups and the host
combines the 16 scalars into the final loss (the "gather/unshard" step).

Within each shard the rows are ordered so that all rows whose group falls
in PSUM chunk 0 (local label < 128) come first, then chunk 1. Each 128-row
tile therefore feeds exactly ONE [128-group x D] PSUM accumulator, halving
the matmul stream count vs. an unsorted layout.

Math per core, per group g:
    cnt[g], sums[g, :] (one-hot matmul), sumsq[g] = sum_i ||x_i||^2
    sum_{i in g} ||x_i - mu_g||^2 = sumsq[g] - ||sums[g]||^2 / cnt[g]

The sums matmul runs in a single bf16 pass: the stationary one-hot is
exact, the moving operand is the truncated-bf16 view of the fp32 rows
(free strided bitcast). Truncation acts as a ~(1 - 2^-8) multiplicative
shrink common to all groups, which cancels in the demog-deviation loss;
the residual noise lands ~5e-4 relative, far under the 2e-2 gate.

sumsq is exact: per-row Square+row-accumulate (rotated between the Scalar
and Vector engines) writes fp32 row norms into a [P, K] staging buffer.
Once per K-tile block, two vector ops split the whole block into bf16
(hi, lo) halves laid out as [hi K | lo K | ones K]; the per-tile 3-column
small matmul then reads a stride-K view [sq_hi, sq_lo, 1] and accumulates
exact [group-sumsq-hi, -lo, count] with the SAME bf16 one-hot as the big
matmul (one LDWEIGHTS dtype, no fp32 weight loads).
"""

import numpy as np

try:
    import concourse.bacc as bacc
except ImportError:  # fresh environment without PYTHONPATH set up
    import sys
    for p in ("/root/.axon_site/_ro/trn_rl_repo", "/opt/trn_rl_repo",
              "/root/.axon_site/_ro/pypackages"):
        if p not in sys.path:
            sys.path.append(p)
    import concourse.bacc as bacc
import concourse.mybir as mybir
import concourse.tile as tile
import concourse.bass_utils as bass_utils

N_CORES = 8
P = 128
D = 512          # feature dim
NL = 256         # labels per core after (demog, label-half) sharding
ND = 4           # demog values
NCH = NL // P    # one-hot chunks of 128 groups
CH = 6           # sample-tiles per feats DMA (1.5 MiB)
K = 8            # tiles per sq-split block
SQ_PATTERN = "svsvs"  # per-tile row-sumsq engine rotation (scalar/vector)

_cache: dict[tuple, object] = {}


def _build(key, debug: bool = False):
    """Compile the SPMD kernel for chunk tile counts (T0, T1)."""
    T0, T1 = key
    T = T0 + T1
    fp32 = mybir.dt.float32
    bf16 = mybir.dt.bfloat16
    Alu = mybir.AluOpType
    Act = mybir.ActivationFunctionType

    nc = bacc.Bacc("TRN2", target_bir_lowering=False, debug=False,
                   enable_asserts=False, num_devices=N_CORES)

    feats_t = nc.dram_tensor("feats_t", [P, T * D], fp32,
                             kind="ExternalInput").ap()
    # labels_t carries [labels | iota table] so the one-hot inputs arrive in
    # one early scalar-queue DMA with no gpsimd dependency
    labels_t = nc.dram_tensor("labels_t", [P, T + NL], fp32,
                              kind="ExternalInput").ap()
    nd_out = nc.dram_tensor("nd", [1, 2], fp32, kind="ExternalOutput").ap()

    chunk_start = (0, T0)
    chunk_stop = (T0 - 1, T - 1)

    with tile.TileContext(nc) as tc:
        with (
            tc.tile_pool(name="const", bufs=1) as constp,
            tc.tile_pool(name="fx", bufs=4) as fxp,
            tc.tile_pool(name="oh16", bufs=K + 3) as oh16p,
            tc.tile_pool(name="scr", bufs=4) as scrp,
            tc.tile_pool(name="post", bufs=1) as postp,
            tc.tile_pool(name="ps", bufs=1, space="PSUM") as psp,
            tc.tile_pool(name="dram", bufs=1, space="DRAM") as dramp,
        ):
            # per-group accumulators; each PSUM accumulation group owns a bank
            ps_sums = [psp.tile([P, D], fp32, tag=f"sums{c}", name=f"sums{c}")
                       for c in range(NCH)]
            ps_small = [psp.tile([P, 3], fp32, tag=f"small{c}",
                                 name=f"small{c}")
                        for c in range(NCH)]

            # chunk schedule: small first chunk so compute starts ASAP; the
            # first NPRE chunks are DMA'd from the scalar queue right at the
            # top of the program (the sync queue sits in the NEFF preamble
            # for ~9us; scalar issues by ~2.5us)
            chunks = [(0, 1)]
            t = 1
            while t < T:
                L = min(CH, T - t)
                chunks.append((t, L))
                t += L
            NPRE = min(3, len(chunks))
            labs = constp.tile([P, T + NL], fp32, tag="labs")
            pre_fx = {}
            for ci in range(NPRE):
                tc0, L = chunks[ci]
                fx = fxp.tile([P, CH * D], fp32, tag="fx")
                nc.scalar.dma_start(out=fx[:, :L * D],
                                    in_=feats_t[:, tc0 * D:(tc0 + L) * D])
                pre_fx[ci] = fx
                if ci == 0:
                    # labels + iota table ride second on the scalar queue
                    nc.scalar.dma_start(out=labs[:], in_=labels_t[:])
            iota32 = labs[:, T:T + NL]

            # rotating sq staging blocks: sqb holds K fp32 row norms, r3b is
            # the bf16 [hi K | lo K | ones K] split (ones written once)
            NBLK = 3
            sqbs, r3bs = [], []
            for b in range(NBLK):
                sqb = constp.tile([P, K], fp32, tag=f"sqb{b}", name=f"sqb{b}")
                r3b = constp.tile([P, 3 * K], bf16, tag=f"r3b{b}",
                                  name=f"r3b{b}")
                nc.vector.memset(r3b[:, 2 * K:3 * K], 1.0)
                sqbs.append(sqb)
                r3bs.append(r3b)
            ones = constp.tile([P, 1], fp32, tag="ones")
            nc.vector.memset(ones[:], 1.0)

            # post-processing tiles (chunk c's slice filled as soon as its
            # accumulation closes, overlapping the other chunk's main loop)
            norm2 = postp.tile([P, NCH], fp32, tag="norm2")
            small = postp.tile([P, 3 * NCH], fp32, tag="small")

            def post_norm2(c):
                scr2 = scrp.tile([P, D], fp32, tag="pscr")
                nc.scalar.activation(scr2[:], ps_sums[c][:], Act.Square,
                                     accum_out=norm2[:, c:c + 1])

            def post_small(c):
                nc.vector.tensor_copy(out=small[:, 3 * c:3 * c + 3],
                                      in_=ps_small[c][:])

            def flush_block(blk, tis, ohs):
                """Split block blk's fp32 row norms into bf16 hi/lo and run
                the delayed small matmuls for its tiles."""
                sqb, r3b = sqbs[blk % NBLK], r3bs[blk % NBLK]
                kk = len(tis)
                hi = sqb[:].bitcast(bf16)[:, 1:2 * kk:2]
                nc.vector.tensor_copy(out=r3b[:, 0:kk], in_=hi)
                nc.vector.tensor_tensor(out=r3b[:, K:K + kk],
                                        in0=sqb[:, 0:kk],
                                        in1=r3b[:, 0:kk], op=Alu.subtract)
                for ti2, oh2 in zip(tis, ohs):
                    c2 = 0 if ti2 < T0 else 1
                    k2 = ti2 % K
                    r3 = r3b[:, k2:k2 + 2 * K + 1:K]  # [sq_hi, sq_lo, 1]
                    nc.tensor.matmul(out=ps_small[c2][:], lhsT=oh2[:],
                                     rhs=r3, start=ti2 in chunk_start,
                                     stop=ti2 in chunk_stop)
                    if ti2 in chunk_stop:
                        post_small(c2)

            # DRAM scratch for warming the output-DMA path mid-loop
            warm_dram = dramp.tile([1, 2], fp32)

            warmed = False
            blk_tis, blk_ohs = [], []
            for ci, (t, L) in enumerate(chunks):
                if ci < NPRE:
                    fx = pre_fx[ci]
                else:
                    fx = fxp.tile([P, CH * D], fp32, tag="fx")
                    nc.sync.dma_start(out=fx[:, :L * D],
                                      in_=feats_t[:, t * D:(t + L) * D])
                if not warmed and t + L >= T - 2 * CH:
                    # keep the output-DMA engine hot for the final nd store
                    nc.sync.dma_start(out=warm_dram[:], in_=labs[:1, :2])
                    warmed = True
                fxb = fx[:].bitcast(bf16)  # [P, CH*2D] uint16-granular view
                for j in range(L):
                    ti = t + j
                    c = 0 if ti < T0 else 1
                    blk = ti // K
                    X = fx[:, j * D:(j + 1) * D]
                    xhi = fxb[:, j * 2 * D + 1:(j + 1) * 2 * D:2]
                    # one-hot of this tile's labels vs the active group chunk
                    oh16 = oh16p.tile([P, P], bf16, tag="oh16")
                    nc.vector.tensor_scalar(
                        out=oh16[:], in0=iota32[:, c * P:(c + 1) * P],
                        scalar1=labs[:, ti:ti + 1], scalar2=None,
                        op0=Alu.is_equal)
                    # exact fp32 row sumsq, engine rotated
                    sqcol = sqbs[blk % NBLK][:, ti % K:ti % K + 1]
                    scr = scrp.tile([P, D], bf16, tag="scr")
                    if SQ_PATTERN[ti % len(SQ_PATTERN)] == "s":
                        nc.scalar.activation(scr[:], X, Act.Square,
                                             accum_out=sqcol)
                    else:
                        nc.vector.scalar_tensor_tensor(
                            out=scr[:], in0=X, scalar=1.0, in1=X,
                            op0=Alu.mult, op1=Alu.mult, accum_out=sqcol)
                    nc.tensor.matmul(out=ps_sums[c][:], lhsT=oh16[:],
                                     rhs=xhi, start=ti in chunk_start,
                                     stop=ti in chunk_stop)
                    if ti in chunk_stop:
                        post_norm2(c)
                    blk_tis.append(ti)
                    blk_ohs.append(oh16)
                    if ti % K == K - 1 or ti == T - 1:
                        flush_block(blk, blk_tis, blk_ohs)
                        blk_tis, blk_ohs = [], []

            # final reduction of this core's 256 groups to (num, den)
            sumsq = postp.tile([P, NCH], fp32, tag="sumsq")
            nc.vector.tensor_tensor(out=sumsq[:], in0=small[:, 0::3],
                                    in1=small[:, 1::3], op=Alu.add)
            cnt = small[:, 2::3]  # [P, NCH]
            safe = postp.tile([P, NCH], fp32, tag="safe")
            nc.vector.tensor_scalar_max(safe[:], cnt, 1.0)
            inv = postp.tile([P, NCH], fp32, tag="inv")
            nc.vector.reciprocal(inv[:], safe[:])
            # grp = (sumsq - norm2 * inv) * inv
            t1 = postp.tile([P, NCH], fp32, tag="t1")
            nc.vector.tensor_tensor(out=t1[:], in0=norm2[:], in1=inv[:],
                                    op=Alu.mult)
            t2 = postp.tile([P, NCH], fp32, tag="t2")
            nc.vector.tensor_tensor(out=t2[:], in0=sumsq[:], in1=t1[:],
                                    op=Alu.subtract)
            grp = postp.tile([P, NCH], fp32, tag="grp")
            nc.vector.tensor_tensor(out=grp[:], in0=t2[:], in1=inv[:],
                                    op=Alu.mult)
            pres = postp.tile([P, NCH], fp32, tag="pres")
            nc.vector.tensor_scalar(out=pres[:], in0=cnt, scalar1=0.0,
                                    scalar2=None, op0=Alu.is_gt)
            # pack [grp*pres | pres]; reduce over this core's groups via PE
            pk = postp.tile([P, 2 * NCH], fp32, tag="pk")
            nc.vector.tensor_tensor(out=pk[:, 0:NCH], in0=grp[:],
                                    in1=pres[:], op=Alu.mult)
            nc.vector.tensor_copy(out=pk[:, NCH:2 * NCH], in_=pres[:])
            ps4 = psp.tile([1, 2 * NCH], fp32, tag="ps4")
            nc.tensor.matmul(out=ps4[:], lhsT=ones[:], rhs=pk[:],
                             start=True, stop=True)
            s4 = postp.tile([1, 2 * NCH], fp32, tag="s4")
            nc.vector.tensor_copy(out=s4[:], in_=ps4[:])
            nd_t = postp.tile([1, 2], fp32, tag="nd_t")
            nc.vector.tensor_reduce(out=nd_t[:, 0:1], in_=s4[:1, 0:NCH],
                                    axis=mybir.AxisListType.X, op=Alu.add)
            nc.vector.tensor_reduce(out=nd_t[:, 1:2], in_=s4[:1, NCH:2 * NCH],
                                    axis=mybir.AxisListType.X, op=Alu.add)
            nc.sync.dma_start(out=nd_out[:], in_=nd_t[:])

    nc.compile()
    return nc


def _shard(feats, labels, demog):
    """Partition rows by (demog, label-half) -> core 2d+h; within each core
    order rows by PSUM chunk (local label < 128 first), padding each chunk
    section to the compile-time tile counts (T0, T1)."""
    half = (labels >= NL).astype(np.int32)
    shard_id = demog * 2 + half
    loc = labels % NL
    chunk = loc // P
    parts = []  # per core: (rows_chunk0, rows_chunk1)
    for s in range(N_CORES):
        in_s = shard_id == s
        parts.append((np.flatnonzero(in_s & (chunk == 0)),
                      np.flatnonzero(in_s & (chunk == 1))))
    T0 = max(1, max(-(-len(p[0]) // P) for p in parts))
    T1 = max(1, max(-(-len(p[1]) // P) for p in parts))
    T = T0 + T1
    S = T * P
    in_maps = []
    for r0, r1 in parts:
        f = np.zeros((S, D), np.float32)
        lab = np.full(S, 999.0, np.float32)  # pad label matches no group
        f[:len(r0)] = feats[r0]
        lab[:len(r0)] = loc[r0]
        f[T0 * P:T0 * P + len(r1)] = feats[r1]
        lab[T0 * P:T0 * P + len(r1)] = loc[r1]
        # [S, D] -> [P, T*D]: partition p holds its rows contiguously so
        # every DMA descriptor is a fat contiguous run
        ft = np.ascontiguousarray(
            f.reshape(T, P, D).transpose(1, 0, 2).reshape(P, T * D))
        lt = np.ascontiguousarray(np.concatenate(
            [lab.reshape(T, P).T,
             np.tile(np.arange(NL, dtype=np.float32), (P, 1))], axis=1))
        in_maps.append({"feats_t": ft, "labels_t": lt})
    return (T0, T1), in_maps


def kernel(feats, labels, demog_labels, _results_out=None):
    feats = np.ascontiguousarray(np.asarray(feats), dtype=np.float32)
    labels = np.asarray(labels).astype(np.int32)
    demog = np.asarray(demog_labels).astype(np.int32)
    assert feats.ndim == 2 and feats.shape[1] == D

    key, in_maps = _shard(feats, labels, demog)
    nc = _cache.get(key)
    if nc is None:
        nc = _cache.setdefault(key, _build(key))
    res = None
    last_exc = None
    for attempt in range(3):
        try:
            res = bass_utils.run_bass_kernel_spmd(
                nc, in_maps, core_ids=list(range(N_CORES)))
            break
        except Exception as e:  # transient axon worker hangups
            last_exc = e
            import time
            time.sleep(10)
    if res is None:
        raise last_exc
    if _results_out is not None:
        _results_out.append(res)
    nds = np.stack([np.asarray(res.results[i]["nd"]).reshape(2)
                    for i in range(N_CORES)])  # [8, 2]
    num = nds[0::2, 0] + nds[1::2, 0]  # per-demog numerator
    den = nds[0::2, 1] + nds[1::2, 1]
    intra = num / np.maximum(den, 1.0)
    loss = np.mean(np.abs(intra - np.mean(intra)))
    return np.float32(loss)
